# revision 31
# baseline (speedup 1.0000x reference)
"""Multi-head attention with random-synthesizer blend + mask, on 8 Trainium2
NeuronCores.  v4: host-fused esyn*mask (one DVE multiply per score tile),
PV lagged one key-chunk behind QK so the PE never stalls on the exp chain,
transposed output projection with per-partition bias, DVE-based norm cast.

Sharding: data-parallel over batch (B=8 -> one batch element per core).

Per-core layouts ([partition, free]):
  - xT (q/k/v): [D, S] fp16, transposed+cast on host; c1 folded into Wq/bq.
  - emsk[h,kc] = (exp((1-alpha)*syn[h].T) * mask.T) fp16 tiles, host-fused.
  - qT/kT: [d_out, s] fp16. v_sb: [s, H*65] fp16 - per head 64 v-dims plus
    one all-ones column, so each PV matmul row 64 yields the softmax sum.
  - Attention per (h,kc): scores_T -> exp (ACT) -> one emsk multiply (DVE)
    -> PV accumulate into pav[0:65].  PV for chunk kc-1 is emitted after
    the projection-fill ops of chunk kc, so the PE stream never waits.
  - Q/K projection chunk hp+1 drained between head pairs (PE stays dense).
  - Normalization: reciprocal_approx_fast on [33,1024] sums, DVE cast to
    fp16, rank-1 ones-matmuls broadcast into PSUM, one DVE multiply.
  - o-proj transposed (Wo chunks stationary, otn moving): out^T[do,s] with
    boeff = bv @ Wo + bo applied as per-partition ACT bias; host transposes
    back.  Output stored fp16 [D, S].
"""

import math
import sys

sys.path.insert(0, "/opt/trn_rl_repo")

import numpy as np

import concourse.tile as tile
import concourse.mybir as mybir
from concourse import bacc
from concourse.bass_utils import run_bass_kernel_spmd

B, S, D, H = 8, 1024, 1024, 16
HD = D // H  # 64
N_CORES = 8
P = 128
SC = S // P  # 8
DC = D // P  # 8
NQ = 512
VW = HD + 1  # 65: v block width incl ones column

f32 = mybir.dt.float32
fp16 = mybir.dt.float16
AF = mybir.ActivationFunctionType
OP = mybir.AluOpType

TRACE = False
TRACE_TMPDIR = None
LAST_RESULTS = None

_CACHE = {}


def _emit(nc, tc, dram):
    w_d = {"q": dram["wq"], "k": dram["wk"], "v": dram["wv"], "o": dram["wo"]}
    x_d = {"q": dram["xq"], "k": dram["xk"], "v": dram["xv"]}
    out_d = dram["out"]

    with (
        tc.tile_pool(name="pers", bufs=1) as pers,
        tc.tile_pool(name="psmm", bufs=1, space="PSUM") as psmm,
        tc.tile_pool(name="psav", bufs=1, space="PSUM") as psav,
    ):
        # ---- constants ---------------------------------------------------
        ones_h = pers.tile([33, P], fp16, tag="ones_h")
        nc.vector.memset(ones_h[:], 1.0)
        bqk_sb = {}
        for nm in ("q", "k"):
            t = pers.tile([P, DC], f32, tag=f"b{nm}", name=f"b{nm}")
            nc.gpsimd.dma_start(out=t[:], in_=dram["b" + nm].rearrange("(c p) -> p c", p=P))
            bqk_sb[nm] = t
        bo_sb = pers.tile([P, DC], f32, tag="bo_sb")
        nc.gpsimd.dma_start(out=bo_sb[:], in_=dram["boeff"].rearrange("(c p) -> p c", p=P))

        # ---- persistent activations --------------------------------------
        qT = [pers.tile([P, S], fp16, tag=f"qT{i}", name=f"qT{i}") for i in range(DC)]
        kT = [pers.tile([P, S], fp16, tag=f"kT{i}", name=f"kT{i}") for i in range(DC)]
        v_sb = [pers.tile([P, H * VW], fp16, tag=f"v{i}", name=f"v{i}")
                for i in range(SC)]
        otn = [pers.tile([P, S], fp16, tag=f"otn{i}", name=f"otn{i}")
               for i in range(DC)]

        def load_tiles(pool, dsrc, prefix, bufs=1, eng=None, chunked=False,
                       halves=False):
            engs = eng if isinstance(eng, (list, tuple)) else [eng or nc.sync]
            tiles = []
            for ci in range(DC):
                t = pool.tile([P, D], fp16, tag=f"{prefix}{ci}", bufs=bufs,
                              name=f"{prefix}{ci}")
                src = dsrc[ci] if chunked else dsrc[ci * P:(ci + 1) * P, :]
                if halves:
                    # half-tile DMAs: the first proj chunk's matmuls start
                    # on finer-grained arrivals instead of 256KB quanta
                    for hh in range(2):
                        engs[(2 * ci + hh) % len(engs)].dma_start(
                            out=t[:, hh * NQ:(hh + 1) * NQ],
                            in_=src[:, hh * NQ:(hh + 1) * NQ])
                else:
                    engs[ci % len(engs)].dma_start(out=t[:], in_=src)
                tiles.append(t)
            return tiles

        def load_w_chunk(pool, nm, do, eng=None):
            # one contiguous DMA: host packs chunk do as [128, 8*128]
            t = pool.tile([P, D], fp16, tag=f"w{nm}c", bufs=2,
                          name=f"w{nm}{do}")
            (eng or nc.sync).dma_start(out=t[:], in_=w_d[nm][do])
            return [t[:, di * P:(di + 1) * P] for di in range(DC)]

        def qk_proj_chunk(nm, wct, xt, dst, do):
            ps = psmm.tile([P, S], f32, tag="mm", bufs=2, name=f"ps{nm}{do}")
            for sq in range(2):
                for di in range(DC):
                    nc.tensor.matmul(
                        ps[:, sq * NQ:(sq + 1) * NQ],
                        wct[di],
                        xt[di][:, sq * NQ:(sq + 1) * NQ],
                        start=(di == 0),
                        stop=(di == DC - 1),
                    )
            nc.scalar.activation(
                out=dst[do][:], in_=ps[:], func=AF.Identity,
                bias=bqk_sb[nm][:, do:do + 1],
            )

        def v_proj_chunk(pool, wt, xt_v, sc):
            nc.gpsimd.memset(v_sb[sc][:], 1.0)
            xt = xt_v[sc]
            xct = [xt[:, di * P:(di + 1) * P] for di in range(DC)]
            ps = psmm.tile([P, S], f32, tag="mm", bufs=2, name=f"psv{sc}")
            for dq in range(2):
                for di in range(DC):
                    nc.tensor.matmul(
                        ps[:, dq * NQ:(dq + 1) * NQ],
                        xct[di],
                        wt[di][:, dq * NQ:(dq + 1) * NQ],
                        start=(di == 0),
                        stop=(di == DC - 1),
                    )
            src = ps[:].rearrange("p (a r) -> p a r", r=HD)
            dst = v_sb[sc][:].rearrange("p (a r) -> p a r", r=VW)
            nc.scalar.copy(out=dst[:, :, 0:HD], in_=src[:, :, :])

        def head(h, ap, spair, vwork=None, filler=None):
            hp, hodd = h // 2, h % 2
            pav = psav.tile([P, S], f32, tag="av", bufs=1, name=f"pav{h}")
            pwork = [None] * SC  # p tiles pending PV

            def pv(kc):
                p = pwork[kc]
                for sq in range(2):
                    nc.tensor.matmul(
                        pav[0:VW, sq * NQ:(sq + 1) * NQ],
                        v_sb[kc][:, h * VW:(h + 1) * VW],
                        p[:, sq * NQ:(sq + 1) * NQ],
                        start=(kc == 0), stop=(kc == SC - 1),
                    )

            for kc in range(SC + 1):
                if kc < SC:
                    if vwork is not None:
                        vwork(kc)
                    emsk_t = ap.tile([P, S], fp16, tag="synT", bufs=6,
                                     name=f"em{h}_{kc}")
                    eng = nc.sync if kc % 2 == 0 else nc.gpsimd
                    eng.dma_start(
                        out=emsk_t[:], in_=dram["emsk"][h, kc * P:(kc + 1) * P, :]
                    )
                    ps = psmm.tile([P, S], f32, tag="mm", bufs=2, name="pss")
                    for sq in range(2):
                        nc.tensor.matmul(
                            ps[:, sq * NQ:(sq + 1) * NQ],
                            kT[hp][hodd * HD:(hodd + 1) * HD, kc * P:(kc + 1) * P],
                            qT[hp][hodd * HD:(hodd + 1) * HD, sq * NQ:(sq + 1) * NQ],
                            start=True, stop=True,
                        )
                    p = ap.tile([P, S], fp16, tag="p", bufs=4, name="p")
                    nc.scalar.activation(out=p[:], in_=ps[:], func=AF.Exp)
                    nc.vector.tensor_tensor(out=p[:], in0=p[:], in1=emsk_t[:],
                                            op=OP.mult)
                    pwork[kc] = p
                if filler is not None and kc < SC:
                    filler()
                if kc > 0:
                    pv(kc - 1)
            # evacuate raw output (ACT) + softmax sums row (DVE)
            nc.scalar.copy(out=otn[hp][hodd * HD:(hodd + 1) * HD, :],
                           in_=pav[0:HD, :])
            nc.vector.tensor_copy(out=spair[32 * hodd:32 * hodd + 1, :],
                                  in_=pav[HD:VW, :])

        def norm(hp, ap, spair, ptag="pmm"):
            # otn[hp] rows 0:64 = head 2hp, 64:128 = head 2hp+1
            # spair rows 1..31 hold 1.0 so the full-tile ops stay finite
            rec = ap.tile([33, S], f32, tag="rec", bufs=1, name=f"rc{hp}")
            nc.vector.reciprocal_approx_fast(out=rec[:], in_=spair[:])
            r16 = ap.tile([33, S], fp16, tag="rec16", bufs=1, name=f"rh{hp}")
            nc.vector.tensor_copy(out=r16[:], in_=rec[:])
            rec16 = [r16[0:1, :], r16[32:33, :]]
            prec = psav.tile([P, S], f32, tag="av", bufs=1, name=f"prc{hp}") \
                if ptag == "av" else \
                psmm.tile([P, S], f32, tag="pmm", bufs=1, name=f"prc{hp}")
            for r in range(2):
                for sq in range(2):
                    nc.tensor.matmul(
                        prec[r * HD:(r + 1) * HD, sq * NQ:(sq + 1) * NQ],
                        ones_h[32 * r:32 * r + 1, 0:HD],
                        rec16[r][:, sq * NQ:(sq + 1) * NQ],
                        start=True, stop=True,
                    )
            nc.vector.tensor_tensor(out=otn[hp][:], in0=otn[hp][:],
                                    in1=prec[:], op=OP.mult)

        # ================= emission ======================================
        with (
            tc.tile_pool(name="projp", bufs=1) as projp,
            tc.tile_pool(name="attn", bufs=1) as ap,
        ):
            # startup loads: weight chunks first; the x tiles feed the PE
            # within ~2us so interleave their DMAs over two DGE queues
            wcq0 = load_w_chunk(projp, "q", 0)
            wck0 = load_w_chunk(projp, "k", 0)
            xq_t = load_tiles(projp, x_d["q"], "xq",
                              eng=[nc.sync, nc.gpsimd], halves=True)
            xk_t = load_tiles(projp, x_d["k"], "xk",
                              eng=[nc.scalar, nc.sync], halves=True)
            wv_t = load_tiles(projp, w_d["v"], "wv", eng=nc.gpsimd)
            xt_v = load_tiles(projp, x_d["v"], "xv", eng=nc.scalar,
                              chunked=True)

            qk_proj_chunk("q", wcq0, xq_t, qT, 0)
            qk_proj_chunk("k", wck0, xk_t, kT, 0)

            # remaining q/k projection chunks drained a few ops per kc slot
            # inside the attention loops (PE stays dense for the HAM clock
            # gate); each chunk's weight DMA is issued one chunk early.
            cw = {}

            def mk_chunk(cid, nm, xt, dst, do):
                key_ps = (cid, "ps")

                def dma():
                    cw[cid] = load_w_chunk(projp, nm, do)

                def mm(sq, di):
                    if key_ps not in cw:
                        cw[key_ps] = psmm.tile([P, S], f32, tag="pmm",
                                               bufs=1, name=f"pp{nm}{do}")
                    nc.tensor.matmul(
                        cw[key_ps][:, sq * NQ:(sq + 1) * NQ],
                        cw[cid][di],
                        xt[di][:, sq * NQ:(sq + 1) * NQ],
                        start=(di == 0),
                        stop=(di == DC - 1),
                    )

                def evac():
                    nc.scalar.activation(
                        out=dst[do][:], in_=cw[key_ps][:], func=AF.Identity,
                        bias=bqk_sb[nm][:, do:do + 1],
                    )

                body = [lambda sq=sq, di=di: mm(sq, di)
                        for sq in range(2) for di in range(DC)]
                body.append(evac)
                return dma, body

            chunks = []
            cid = 0
            for do in range(1, DC):
                for nm, xt, dst in (("q", xq_t, qT), ("k", xk_t, kT)):
                    chunks.append((cid, mk_chunk(cid, nm, xt, dst, do)))
                    cid += 1
            # stagger: dma(c0), dma(c1), body(c0), dma(c2), body(c1), ...
            proj_work = []  # list of (chunk_id_done_after_op, fn)
            if chunks:
                proj_work.append((-1, chunks[0][1][0]))
                for i, (ci, (dma, body)) in enumerate(chunks):
                    if i + 1 < len(chunks):
                        proj_work.append((-1, chunks[i + 1][1][0]))
                    for b_idx, fn in enumerate(body):
                        proj_work.append(
                            (ci if b_idx == len(body) - 1 else -1, fn))
            proj_work.reverse()  # pop() from the end

            done_cid = [-1]

            def pop_one():
                ci, fn = proj_work.pop()
                fn()
                if ci >= 0:
                    done_cid[0] = ci

            def drain(n):
                def f():
                    for _ in range(n):
                        if proj_work:
                            pop_one()
                return f

            def force_until(cid_needed):
                while proj_work and done_cid[0] < cid_needed:
                    pop_one()

            wt_o = None
            spairs = {}

            # fill budget: 14 chunks x 18 ops = 252 ops must last through
            # pair 7 (36/pair = exactly the deadline rate), so drain at the
            # deadline rate and let force_until supply any shortfall one
            # head ahead of each pair's first QK.
            o_early = {}

            def o_mm(ps, do, sq, di):
                nc.tensor.matmul(
                    ps[:, sq * NQ:(sq + 1) * NQ],
                    wt_o[do][:, di * P:(di + 1) * P],
                    otn[di][:, sq * NQ:(sq + 1) * NQ],
                    start=(di == 0), stop=(di == DC - 1),
                )

            for hp in range(DC):
                if hp == 1:
                    wt_o = load_tiles(projp, w_d["o"], "wo", chunked=True)
                spair = ap.tile([33, S], f32, tag="spair", bufs=2,
                                name=f"sp{hp}")
                spairs[hp] = spair
                nc.gpsimd.memset(spair[:], 1.0)
                if hp == 0:
                    head(0, ap, spair,
                         vwork=lambda kc: v_proj_chunk(projp, wv_t, xt_v, kc),
                         filler=drain(2))
                else:
                    # chunks q/k(hp) must be fully emitted before this pair
                    force_until(2 * hp - 1)
                    head(2 * hp, ap, spair, filler=drain(2))
                if hp > 0:
                    # deferred: previous pair's normalization hides behind
                    # this pair's attention stream
                    norm(hp - 1, ap, spairs.pop(hp - 1))
                if hp == DC - 1:
                    # early o-proj chunk 0 partials (otn[0..5] are final):
                    # keeps the PE dense through the fill-less last pair
                    o_early[0] = psmm.tile([P, S], f32, tag="pmm", bufs=1,
                                           name="pso0")
                    for sq in range(2):
                        for di in range(DC - 1):
                            o_mm(o_early[0], 0, sq, di)
                # next pair's chunks complete one head before its deadline
                # so the evacuations never gate its first QK matmuls
                if hp + 1 < DC:
                    force_until(2 * (hp + 1) - 1)
                head(2 * hp + 1, ap, spair, filler=drain(2))
            while proj_work:
                pop_one()
            norm(DC - 1, ap, spairs.pop(DC - 1), ptag="av")

            # ============= output projection (transposed) ================
            # out^T[do*P+p, s] = sum_d Wo[d, do*P+p] * otn[d, s] + boeff
            # chunk 0 accumulated early; chunks 1-2 defer their di=6,7
            # matmuls + evac so the PE is never gated on norm(DC-1)
            def o_finish(ps, do):
                osb = ap.tile([P, S], fp16, tag="osb", bufs=2, name="osb")
                nc.scalar.activation(
                    out=osb[:], in_=ps[:], func=AF.Identity,
                    bias=bo_sb[:, do:do + 1],
                )
                nc.sync.dma_start(out=out_d[do * P:(do + 1) * P, :], in_=osb[:])

            ps_head = {}
            for do in (1, 2):
                ps_head[do] = psmm.tile([P, S], f32, tag="mm", bufs=2,
                                        name=f"pso{do}")
                for sq in range(2):
                    for di in range(DC - 1):
                        o_mm(ps_head[do], do, sq, di)
            for do in (0, 1, 2):
                ps = o_early[0] if do == 0 else ps_head[do]
                for sq in range(2):
                    o_mm(ps, do, sq, DC - 1)
                o_finish(ps, do)
            for do in range(3, DC):
                ps = psmm.tile([P, S], f32, tag="mm", bufs=2, name=f"pso{do}")
                for sq in range(2):
                    for di in range(DC):
                        o_mm(ps, do, sq, di)
                o_finish(ps, do)


def _build():
    nc = bacc.Bacc("TRN2", debug=False)
    dram = {
        "xq": nc.declare_dram_parameter("xq", [D, S], fp16, isOutput=False),
        "xk": nc.declare_dram_parameter("xk", [D, S], fp16, isOutput=False),
        "xv": nc.declare_dram_parameter("xv", [SC, P, D], fp16, isOutput=False),
        "wq": nc.declare_dram_parameter("wq", [DC, P, D], fp16, isOutput=False),
        "wk": nc.declare_dram_parameter("wk", [DC, P, D], fp16, isOutput=False),
        "wv": nc.declare_dram_parameter("wv", [D, D], fp16, isOutput=False),
        "wo": nc.declare_dram_parameter("wo", [DC, P, D], fp16, isOutput=False),
        "bq": nc.declare_dram_parameter("bq", [D], f32, isOutput=False),
        "bk": nc.declare_dram_parameter("bk", [D], f32, isOutput=False),
        "boeff": nc.declare_dram_parameter("boeff", [D], f32, isOutput=False),
        "emsk": nc.declare_dram_parameter("emsk", [H, S, S], fp16, isOutput=False),
        "out": nc.declare_dram_parameter("out", [D, S], fp16, isOutput=True),
    }
    with tile.TileContext(nc) as tc:
        _emit(nc, tc, dram)
    nc.compile()
    return nc


def _prep(inputs):
    q = np.asarray(inputs["query"], np.float32)
    k = np.asarray(inputs["key"], np.float32)
    v = np.asarray(inputs["value"], np.float32)
    msk = np.asarray(inputs["mask"], np.int32)
    ws = {nm: np.asarray(inputs["W" + nm], np.float32) for nm in "qkvo"}
    bs = {nm: np.asarray(inputs["b" + nm], np.float32) for nm in "qkvo"}
    alpha = 1.0 / (1.0 + math.exp(-float(np.asarray(inputs["alpha_param"]).ravel()[0])))
    c1 = alpha / math.sqrt(HD)
    c2 = 1.0 - alpha

    # esynT[h][k, q] = exp(c2 * syn[h][q, k]) in fp16
    esynT = np.exp(
        c2 * np.asarray(inputs["syn_scores"], np.float32)[:, :S, :S].transpose(0, 2, 1)
    ).astype(np.float16)
    boeff = (bs["v"].astype(np.float64) @ ws["o"].astype(np.float64)
             + bs["o"]).astype(np.float32)

    def chunk_pack(w):
        # [do, p, di*P + c] = w[di*P + p, do*P + c]
        w4 = w.reshape(DC, P, DC, P)          # [di, p, do, c]
        return np.ascontiguousarray(
            w4.transpose(2, 1, 0, 3).reshape(DC, P, D))

    common = {
        "wq": chunk_pack((c1 * ws["q"]).astype(np.float16)),
        "wk": chunk_pack(ws["k"].astype(np.float16)),
        "wv": ws["v"].astype(np.float16),
        "wo": chunk_pack(ws["o"].astype(np.float16)),
        "bq": (c1 * bs["q"]).astype(np.float32),
        "bk": bs["k"],
        "boeff": boeff,
    }
    in_maps = []
    for b in range(B):
        m = dict(common)
        m["xq"] = np.ascontiguousarray(q[b].T.astype(np.float16))
        m["xk"] = np.ascontiguousarray(k[b].T.astype(np.float16))
        m["xv"] = chunk_pack(v[b].T.astype(np.float16))
        # emsk[h][k, q] = esynT[h][k, q] * mask[b][q, k]
        mTb = np.ascontiguousarray(msk[b].T).astype(np.float16)
        m["emsk"] = esynT * mTb[None, :, :]
        in_maps.append(m)
    return in_maps


def kernel(**inputs):
    global LAST_RESULTS
    if "nc" not in _CACHE:
        _CACHE["nc"] = _build()
    nc = _CACHE["nc"]
    in_maps = _prep(inputs)

    kwargs = {}
    if TRACE:
        kwargs["trace"] = True
        if TRACE_TMPDIR:
            kwargs["tmpdir"] = TRACE_TMPDIR
    res = run_bass_kernel_spmd(nc, in_maps, core_ids=list(range(N_CORES)), **kwargs)
    LAST_RESULTS = res
    return np.stack(
        [res.results[b]["out"].astype(np.float32).T for b in range(B)], axis=0
    )


# revision 32
# speedup vs baseline: 1.0206x; 1.0206x over previous
"""Multi-head attention with random-synthesizer blend + mask, on 8 Trainium2
NeuronCores.  v4: host-fused esyn*mask (one DVE multiply per score tile),
PV lagged one key-chunk behind QK so the PE never stalls on the exp chain,
transposed output projection with per-partition bias, DVE-based norm cast.

Sharding: data-parallel over batch (B=8 -> one batch element per core).

Per-core layouts ([partition, free]):
  - xT (q/k/v): [D, S] fp16, transposed+cast on host; c1 folded into Wq/bq.
  - emsk[h,kc] = (exp((1-alpha)*syn[h].T) * mask.T) fp16 tiles, host-fused.
  - qT/kT: [d_out, s] fp16. v_sb: [s, H*65] fp16 - per head 64 v-dims plus
    one all-ones column, so each PV matmul row 64 yields the softmax sum.
  - Attention per (h,kc): scores_T -> exp (ACT) -> one emsk multiply (DVE)
    -> PV accumulate into pav[0:65].  PV for chunk kc-1 is emitted after
    the projection-fill ops of chunk kc, so the PE stream never waits.
  - Q/K projection chunk hp+1 drained between head pairs (PE stays dense).
  - Normalization: reciprocal_approx_fast on [33,1024] sums, DVE cast to
    fp16, rank-1 ones-matmuls broadcast into PSUM, one DVE multiply.
  - o-proj transposed (Wo chunks stationary, otn moving): out^T[do,s] with
    boeff = bv @ Wo + bo applied as per-partition ACT bias; host transposes
    back.  Output stored fp16 [D, S].
"""

import math
import sys

sys.path.insert(0, "/opt/trn_rl_repo")

import numpy as np

import concourse.tile as tile
import concourse.mybir as mybir
from concourse import bacc
from concourse.bass_utils import run_bass_kernel_spmd

B, S, D, H = 8, 1024, 1024, 16
HD = D // H  # 64
N_CORES = 8
P = 128
SC = S // P  # 8
DC = D // P  # 8
NQ = 512
VW = HD + 1  # 65: v block width incl ones column

f32 = mybir.dt.float32
fp16 = mybir.dt.float16
AF = mybir.ActivationFunctionType
OP = mybir.AluOpType

TRACE = False
TRACE_TMPDIR = None
LAST_RESULTS = None

_CACHE = {}


def _emit(nc, tc, dram):
    w_d = {"q": dram["wq"], "k": dram["wk"], "v": dram["wv"], "o": dram["wo"]}
    x_d = {"q": dram["xq"], "k": dram["xk"], "v": dram["xv"]}
    out_d = dram["out"]

    with (
        tc.tile_pool(name="pers", bufs=1) as pers,
        tc.tile_pool(name="psmm", bufs=1, space="PSUM") as psmm,
        tc.tile_pool(name="psav", bufs=1, space="PSUM") as psav,
    ):
        # ---- constants ---------------------------------------------------
        ones_h = pers.tile([33, P], fp16, tag="ones_h")
        nc.vector.memset(ones_h[:], 1.0)
        bqk_sb = {}
        for nm in ("q", "k"):
            t = pers.tile([P, DC], f32, tag=f"b{nm}", name=f"b{nm}")
            nc.gpsimd.dma_start(out=t[:], in_=dram["b" + nm].rearrange("(c p) -> p c", p=P))
            bqk_sb[nm] = t
        bo_sb = pers.tile([P, DC], f32, tag="bo_sb")
        nc.gpsimd.dma_start(out=bo_sb[:], in_=dram["boeff"].rearrange("(c p) -> p c", p=P))

        # ---- persistent activations --------------------------------------
        qT = [pers.tile([P, S], fp16, tag=f"qT{i}", name=f"qT{i}") for i in range(DC)]
        kT = [pers.tile([P, S], fp16, tag=f"kT{i}", name=f"kT{i}") for i in range(DC)]
        v_sb = [pers.tile([P, H * VW], fp16, tag=f"v{i}", name=f"v{i}")
                for i in range(SC)]
        otn = [pers.tile([P, S], fp16, tag=f"otn{i}", name=f"otn{i}")
               for i in range(DC)]

        def load_tiles(pool, dsrc, prefix, bufs=1, eng=None, chunked=False,
                       halves=False):
            engs = eng if isinstance(eng, (list, tuple)) else [eng or nc.sync]
            tiles = []
            for ci in range(DC):
                t = pool.tile([P, D], fp16, tag=f"{prefix}{ci}", bufs=bufs,
                              name=f"{prefix}{ci}")
                src = dsrc[ci] if chunked else dsrc[ci * P:(ci + 1) * P, :]
                if halves:
                    # half-tile DMAs: the first proj chunk's matmuls start
                    # on finer-grained arrivals instead of 256KB quanta
                    for hh in range(2):
                        engs[(2 * ci + hh) % len(engs)].dma_start(
                            out=t[:, hh * NQ:(hh + 1) * NQ],
                            in_=src[:, hh * NQ:(hh + 1) * NQ])
                else:
                    engs[ci % len(engs)].dma_start(out=t[:], in_=src)
                tiles.append(t)
            return tiles

        def load_w_chunk(pool, nm, do, eng=None):
            # one contiguous DMA: host packs chunk do as [128, 8*128]
            t = pool.tile([P, D], fp16, tag=f"w{nm}c", bufs=2,
                          name=f"w{nm}{do}")
            (eng or nc.sync).dma_start(out=t[:], in_=w_d[nm][do])
            return [t[:, di * P:(di + 1) * P] for di in range(DC)]

        def qk_proj_chunk(nm, wct, xt, dst, do):
            ps = psmm.tile([P, S], f32, tag="mm", bufs=2, name=f"ps{nm}{do}")
            for sq in range(2):
                for di in range(DC):
                    nc.tensor.matmul(
                        ps[:, sq * NQ:(sq + 1) * NQ],
                        wct[di],
                        xt[di][:, sq * NQ:(sq + 1) * NQ],
                        start=(di == 0),
                        stop=(di == DC - 1),
                    )
            nc.scalar.activation(
                out=dst[do][:], in_=ps[:], func=AF.Identity,
                bias=bqk_sb[nm][:, do:do + 1],
            )

        def v_proj_chunk(pool, wt, xt_v, sc):
            nc.gpsimd.memset(v_sb[sc][:], 1.0)
            xt = xt_v[sc]
            xct = [xt[:, di * P:(di + 1) * P] for di in range(DC)]
            ps = psmm.tile([P, S], f32, tag="mm", bufs=2, name=f"psv{sc}")
            for dq in range(2):
                for di in range(DC):
                    nc.tensor.matmul(
                        ps[:, dq * NQ:(dq + 1) * NQ],
                        xct[di],
                        wt[di][:, dq * NQ:(dq + 1) * NQ],
                        start=(di == 0),
                        stop=(di == DC - 1),
                    )
            src = ps[:].rearrange("p (a r) -> p a r", r=HD)
            dst = v_sb[sc][:].rearrange("p (a r) -> p a r", r=VW)
            nc.scalar.copy(out=dst[:, :, 0:HD], in_=src[:, :, :])

        def head(h, ap, spair, vwork=None, filler=None):
            hp, hodd = h // 2, h % 2
            pav = psav.tile([P, S], f32, tag="av", bufs=1, name=f"pav{h}")
            pwork = [None] * SC  # p tiles pending PV

            def pv(kc):
                p = pwork[kc]
                for sq in range(2):
                    nc.tensor.matmul(
                        pav[0:VW, sq * NQ:(sq + 1) * NQ],
                        v_sb[kc][:, h * VW:(h + 1) * VW],
                        p[:, sq * NQ:(sq + 1) * NQ],
                        start=(kc == 0), stop=(kc == SC - 1),
                    )

            for kc in range(SC + 1):
                if kc < SC:
                    if vwork is not None:
                        vwork(kc)
                    emsk_t = ap.tile([P, S], fp16, tag="synT", bufs=6,
                                     name=f"em{h}_{kc}")
                    eng = nc.sync if kc % 2 == 0 else nc.gpsimd
                    eng.dma_start(
                        out=emsk_t[:], in_=dram["emsk"][h, kc * P:(kc + 1) * P, :]
                    )
                    ps = psmm.tile([P, S], f32, tag="mm", bufs=2, name="pss")
                    for sq in range(2):
                        nc.tensor.matmul(
                            ps[:, sq * NQ:(sq + 1) * NQ],
                            kT[hp][hodd * HD:(hodd + 1) * HD, kc * P:(kc + 1) * P],
                            qT[hp][hodd * HD:(hodd + 1) * HD, sq * NQ:(sq + 1) * NQ],
                            start=True, stop=True,
                        )
                    p = ap.tile([P, S], fp16, tag="p", bufs=4, name="p")
                    nc.scalar.activation(out=p[:], in_=ps[:], func=AF.Exp)
                    nc.vector.tensor_tensor(out=p[:], in0=p[:], in1=emsk_t[:],
                                            op=OP.mult)
                    pwork[kc] = p
                if filler is not None:
                    filler()
                if kc > 0:
                    pv(kc - 1)
            # evacuate raw output (ACT) + softmax sums row (DVE)
            nc.scalar.copy(out=otn[hp][hodd * HD:(hodd + 1) * HD, :],
                           in_=pav[0:HD, :])
            nc.vector.tensor_copy(out=spair[32 * hodd:32 * hodd + 1, :],
                                  in_=pav[HD:VW, :])

        def norm(hp, ap, spair, ptag="pmm"):
            # otn[hp] rows 0:64 = head 2hp, 64:128 = head 2hp+1
            # spair rows 1..31 hold 1.0 so the full-tile ops stay finite
            rec = ap.tile([33, S], f32, tag="rec", bufs=1, name=f"rc{hp}")
            nc.vector.reciprocal_approx_fast(out=rec[:], in_=spair[:])
            r16 = ap.tile([33, S], fp16, tag="rec16", bufs=1, name=f"rh{hp}")
            nc.vector.tensor_copy(out=r16[:], in_=rec[:])
            rec16 = [r16[0:1, :], r16[32:33, :]]
            prec = psav.tile([P, S], f32, tag="av", bufs=1, name=f"prc{hp}") \
                if ptag == "av" else \
                psmm.tile([P, S], f32, tag="pmm", bufs=1, name=f"prc{hp}")
            for r in range(2):
                for sq in range(2):
                    nc.tensor.matmul(
                        prec[r * HD:(r + 1) * HD, sq * NQ:(sq + 1) * NQ],
                        ones_h[32 * r:32 * r + 1, 0:HD],
                        rec16[r][:, sq * NQ:(sq + 1) * NQ],
                        start=True, stop=True,
                    )
            nc.vector.tensor_tensor(out=otn[hp][:], in0=otn[hp][:],
                                    in1=prec[:], op=OP.mult)

        # ================= emission ======================================
        with (
            tc.tile_pool(name="projp", bufs=1) as projp,
            tc.tile_pool(name="attn", bufs=1) as ap,
        ):
            # startup loads: weight chunks first; the x tiles feed the PE
            # within ~2us so interleave their DMAs over two DGE queues
            wcq0 = load_w_chunk(projp, "q", 0)
            wck0 = load_w_chunk(projp, "k", 0)
            xq_t = load_tiles(projp, x_d["q"], "xq",
                              eng=[nc.sync, nc.gpsimd], halves=True)
            xk_t = load_tiles(projp, x_d["k"], "xk",
                              eng=[nc.scalar, nc.sync], halves=True)
            wv_t = load_tiles(projp, w_d["v"], "wv", eng=nc.gpsimd)
            xt_v = load_tiles(projp, x_d["v"], "xv", eng=nc.scalar,
                              chunked=True)

            qk_proj_chunk("q", wcq0, xq_t, qT, 0)
            qk_proj_chunk("k", wck0, xk_t, kT, 0)

            # remaining q/k projection chunks drained a few ops per kc slot
            # inside the attention loops (PE stays dense for the HAM clock
            # gate); each chunk's weight DMA is issued one chunk early.
            cw = {}

            def mk_chunk(cid, nm, xt, dst, do):
                key_ps = (cid, "ps")

                def dma():
                    cw[cid] = load_w_chunk(projp, nm, do)

                def mm(sq, di):
                    if key_ps not in cw:
                        cw[key_ps] = psmm.tile([P, S], f32, tag="pmm",
                                               bufs=1, name=f"pp{nm}{do}")
                    nc.tensor.matmul(
                        cw[key_ps][:, sq * NQ:(sq + 1) * NQ],
                        cw[cid][di],
                        xt[di][:, sq * NQ:(sq + 1) * NQ],
                        start=(di == 0),
                        stop=(di == DC - 1),
                    )

                def evac():
                    nc.scalar.activation(
                        out=dst[do][:], in_=cw[key_ps][:], func=AF.Identity,
                        bias=bqk_sb[nm][:, do:do + 1],
                    )

                body = [lambda sq=sq, di=di: mm(sq, di)
                        for sq in range(2) for di in range(DC)]
                body.append(evac)
                return dma, body

            chunks = []
            cid = 0
            for do in range(1, DC):
                for nm, xt, dst in (("q", xq_t, qT), ("k", xk_t, kT)):
                    chunks.append((cid, mk_chunk(cid, nm, xt, dst, do)))
                    cid += 1
            # stagger: dma(c0), dma(c1), body(c0), dma(c2), body(c1), ...
            proj_work = []  # list of (chunk_id_done_after_op, fn)
            if chunks:
                proj_work.append((-1, chunks[0][1][0]))
                for i, (ci, (dma, body)) in enumerate(chunks):
                    if i + 1 < len(chunks):
                        proj_work.append((-1, chunks[i + 1][1][0]))
                    for b_idx, fn in enumerate(body):
                        proj_work.append(
                            (ci if b_idx == len(body) - 1 else -1, fn))
            proj_work.reverse()  # pop() from the end

            done_cid = [-1]

            def pop_one():
                ci, fn = proj_work.pop()
                fn()
                if ci >= 0:
                    done_cid[0] = ci

            def drain(n):
                def f():
                    for _ in range(n):
                        if proj_work:
                            pop_one()
                return f

            def force_until(cid_needed):
                while proj_work and done_cid[0] < cid_needed:
                    pop_one()

            wt_o = None
            spairs = {}

            # fill budget: 14 chunks x 18 ops = 252 ops must last through
            # pair 7 (36/pair = exactly the deadline rate), so drain at the
            # deadline rate and let force_until supply any shortfall one
            # head ahead of each pair's first QK.
            o_early = {}

            def o_mm(ps, do, sq, di):
                nc.tensor.matmul(
                    ps[:, sq * NQ:(sq + 1) * NQ],
                    wt_o[do][:, di * P:(di + 1) * P],
                    otn[di][:, sq * NQ:(sq + 1) * NQ],
                    start=(di == 0), stop=(di == DC - 1),
                )

            for hp in range(DC):
                if hp == 1:
                    wt_o = load_tiles(projp, w_d["o"], "wo", chunked=True)
                spair = ap.tile([33, S], f32, tag="spair", bufs=2,
                                name=f"sp{hp}")
                spairs[hp] = spair
                nc.gpsimd.memset(spair[:], 1.0)
                if hp == 0:
                    head(0, ap, spair,
                         vwork=lambda kc: v_proj_chunk(projp, wv_t, xt_v, kc),
                         filler=drain(2))
                else:
                    # chunks q/k(hp) must be fully emitted before this pair
                    force_until(2 * hp - 1)
                    head(2 * hp, ap, spair, filler=drain(2))
                if hp > 0:
                    # deferred: previous pair's normalization hides behind
                    # this pair's attention stream
                    norm(hp - 1, ap, spairs.pop(hp - 1))
                if hp == DC - 1:
                    # early o-proj chunk 0 partials (otn[0..5] are final):
                    # keeps the PE dense through the fill-less last pair
                    o_early[0] = psmm.tile([P, S], f32, tag="pmm", bufs=1,
                                           name="pso0")
                    for sq in range(2):
                        for di in range(DC - 1):
                            o_mm(o_early[0], 0, sq, di)
                # next pair's chunks complete one head before its deadline
                # so the evacuations never gate its first QK matmuls
                if hp + 1 < DC:
                    force_until(2 * (hp + 1) - 1)
                head(2 * hp + 1, ap, spair,
                     filler=drain(4 if hp == 0 else 2))
            while proj_work:
                pop_one()
            norm(DC - 1, ap, spairs.pop(DC - 1), ptag="av")

            # ============= output projection (transposed) ================
            # out^T[do*P+p, s] = sum_d Wo[d, do*P+p] * otn[d, s] + boeff
            # chunk 0 accumulated early; chunks 1-2 defer their di=6,7
            # matmuls + evac so the PE is never gated on norm(DC-1)
            def o_finish(ps, do):
                osb = ap.tile([P, S], fp16, tag="osb", bufs=2, name="osb")
                nc.scalar.activation(
                    out=osb[:], in_=ps[:], func=AF.Identity,
                    bias=bo_sb[:, do:do + 1],
                )
                nc.sync.dma_start(out=out_d[do * P:(do + 1) * P, :], in_=osb[:])

            ps_head = {}
            for do in (1, 2):
                ps_head[do] = psmm.tile([P, S], f32, tag="mm", bufs=2,
                                        name=f"pso{do}")
                for sq in range(2):
                    for di in range(DC - 1):
                        o_mm(ps_head[do], do, sq, di)
            for do in (0, 1, 2):
                ps = o_early[0] if do == 0 else ps_head[do]
                for sq in range(2):
                    o_mm(ps, do, sq, DC - 1)
                o_finish(ps, do)
            for do in range(3, DC):
                ps = psmm.tile([P, S], f32, tag="mm", bufs=2, name=f"pso{do}")
                for sq in range(2):
                    for di in range(DC):
                        o_mm(ps, do, sq, di)
                o_finish(ps, do)


def _build():
    nc = bacc.Bacc("TRN2", debug=False)
    dram = {
        "xq": nc.declare_dram_parameter("xq", [D, S], fp16, isOutput=False),
        "xk": nc.declare_dram_parameter("xk", [D, S], fp16, isOutput=False),
        "xv": nc.declare_dram_parameter("xv", [SC, P, D], fp16, isOutput=False),
        "wq": nc.declare_dram_parameter("wq", [DC, P, D], fp16, isOutput=False),
        "wk": nc.declare_dram_parameter("wk", [DC, P, D], fp16, isOutput=False),
        "wv": nc.declare_dram_parameter("wv", [D, D], fp16, isOutput=False),
        "wo": nc.declare_dram_parameter("wo", [DC, P, D], fp16, isOutput=False),
        "bq": nc.declare_dram_parameter("bq", [D], f32, isOutput=False),
        "bk": nc.declare_dram_parameter("bk", [D], f32, isOutput=False),
        "boeff": nc.declare_dram_parameter("boeff", [D], f32, isOutput=False),
        "emsk": nc.declare_dram_parameter("emsk", [H, S, S], fp16, isOutput=False),
        "out": nc.declare_dram_parameter("out", [D, S], fp16, isOutput=True),
    }
    with tile.TileContext(nc) as tc:
        _emit(nc, tc, dram)
    nc.compile()
    return nc


def _prep(inputs):
    q = np.asarray(inputs["query"], np.float32)
    k = np.asarray(inputs["key"], np.float32)
    v = np.asarray(inputs["value"], np.float32)
    msk = np.asarray(inputs["mask"], np.int32)
    ws = {nm: np.asarray(inputs["W" + nm], np.float32) for nm in "qkvo"}
    bs = {nm: np.asarray(inputs["b" + nm], np.float32) for nm in "qkvo"}
    alpha = 1.0 / (1.0 + math.exp(-float(np.asarray(inputs["alpha_param"]).ravel()[0])))
    c1 = alpha / math.sqrt(HD)
    c2 = 1.0 - alpha

    # esynT[h][k, q] = exp(c2 * syn[h][q, k]) in fp16
    esynT = np.exp(
        c2 * np.asarray(inputs["syn_scores"], np.float32)[:, :S, :S].transpose(0, 2, 1)
    ).astype(np.float16)
    boeff = (bs["v"].astype(np.float64) @ ws["o"].astype(np.float64)
             + bs["o"]).astype(np.float32)

    def chunk_pack(w):
        # [do, p, di*P + c] = w[di*P + p, do*P + c]
        w4 = w.reshape(DC, P, DC, P)          # [di, p, do, c]
        return np.ascontiguousarray(
            w4.transpose(2, 1, 0, 3).reshape(DC, P, D))

    common = {
        "wq": chunk_pack((c1 * ws["q"]).astype(np.float16)),
        "wk": chunk_pack(ws["k"].astype(np.float16)),
        "wv": ws["v"].astype(np.float16),
        "wo": chunk_pack(ws["o"].astype(np.float16)),
        "bq": (c1 * bs["q"]).astype(np.float32),
        "bk": bs["k"],
        "boeff": boeff,
    }
    in_maps = []
    for b in range(B):
        m = dict(common)
        m["xq"] = np.ascontiguousarray(q[b].T.astype(np.float16))
        m["xk"] = np.ascontiguousarray(k[b].T.astype(np.float16))
        m["xv"] = chunk_pack(v[b].T.astype(np.float16))
        # emsk[h][k, q] = esynT[h][k, q] * mask[b][q, k]
        mTb = np.ascontiguousarray(msk[b].T).astype(np.float16)
        m["emsk"] = esynT * mTb[None, :, :]
        in_maps.append(m)
    return in_maps


def kernel(**inputs):
    global LAST_RESULTS
    if "nc" not in _CACHE:
        _CACHE["nc"] = _build()
    nc = _CACHE["nc"]
    in_maps = _prep(inputs)

    kwargs = {}
    if TRACE:
        kwargs["trace"] = True
        if TRACE_TMPDIR:
            kwargs["tmpdir"] = TRACE_TMPDIR
    res = run_bass_kernel_spmd(nc, in_maps, core_ids=list(range(N_CORES)), **kwargs)
    LAST_RESULTS = res
    return np.stack(
        [res.results[b]["out"].astype(np.float32).T for b in range(B)], axis=0
    )
